# revision 1
# baseline (speedup 1.0000x reference)
"""DCNv3_C Trainium2 Bass kernel.

8-core data parallelism over the batch (one image per NeuronCore).
Per core: 1x1 conv -> value proj -> depthwise 3x3 (block-diag matmuls)
-> LN+gelu -> offset/mask proj -> softmax -> dense 5x5 "hat" sampling
weights -> 25-bin weighted window sum (DVE scalar_tensor_tensor)
-> output proj.

DCNv3 bilinear sampling is rewritten exactly (for |offset|<=1) as a 5x5
locally-connected window:
  acc[s,g,c] = sum_{dy,dx in [-2,2]} DW[s,g,dy,dx] * VP[s+(dy,dx), g, c]
  DW[s,g,dy,dx] = sum_p mask_p * hat(gy_p+offy_p-dy) * hat(gx_p+offx_p-dx)
with hat(t)=max(0,1-|t|) and VP the value map zero-padded by 2.
"""

import numpy as np

N, C_IN, C, H, W = 8, 192, 256, 64, 64
G, K, PAD = 4, 3, 1
GC = C // G          # 64
P = K * K            # 9
S = H * W            # 4096
NCORES = 8

_CACHE = {}
TRACE = False
_LAST_EXEC_NS = None


def _host_consts():
    # p = a*3+b with grid_x = a-1 (slowest), grid_y = b-1
    gx = np.repeat(np.arange(3) - 1, 3)
    gy = np.tile(np.arange(3) - 1, 3)
    # p-sum selection matrices, one per (xb, yb): [36, 100]
    # row (g, p) -> col g*25 + d, d = (dy+2)*5 + (dx+2)
    Smats = np.zeros((3, 3, 36, 100), np.float32)
    for xb in range(3):
        for yb in range(3):
            for g in range(G):
                for p_ in range(P):
                    dy = gy[p_] + yb - 1
                    dx = gx[p_] + xb - 1
                    d = (dy + 2) * 5 + (dx + 2)
                    Smats[xb, yb, g * 9 + p_, g * 25 + d] = 1.0
    E9 = np.zeros((36, 4), np.float32)     # per-group sums
    E9T = np.zeros((4, 36), np.float32)    # per-group broadcast
    for g in range(G):
        E9[g * 9:(g + 1) * 9, g] = 1.0
        E9T[g, g * 9:(g + 1) * 9] = 1.0
    return Smats, E9, E9T


def _prep_weights(inp):
    import ml_dtypes as _mldw
    w = {}
    w['wc'] = np.ascontiguousarray(inp['conv_w'].T).astype(np.float32)   # [192,256]
    w['bc'] = inp['conv_b'].reshape(C, 1).astype(np.float32)
    w['win'] = np.ascontiguousarray(inp['in_w'].T).astype(_mldw.bfloat16)  # [c,o]
    w['inb'] = np.asarray(inp['in_b'], np.float32)
    # depthwise diag weights, partition-major: [128, 9, 2, 128]
    import ml_dtypes as _mld0
    dwd = np.zeros((128, 9, 2, 128), np.float32)
    dw = inp['dw_w'].reshape(C, 9)
    for tap in range(9):
        for mt in range(2):
            for i in range(128):
                dwd[i, tap, mt, i] = dw[mt * 128 + i, tap]
    w['dwd'] = dwd.astype(_mld0.bfloat16)
    w['bdw'] = inp['dw_b'].reshape(C, 1).astype(np.float32)
    w['ln_g'] = inp['ln_g'].reshape(C, 1).astype(np.float32)
    w['ln_b'] = inp['ln_b'].reshape(C, 1).astype(np.float32)
    # offset/mask projections: wox/woy/wmk [256, 36] lhsT, col = g*9+p
    wox = np.zeros((C, 36), np.float32)
    woy = np.zeros((C, 36), np.float32)
    box = np.zeros((36, 1), np.float32)
    boy = np.zeros((36, 1), np.float32)
    ow, ob = np.asarray(inp['off_w'], np.float32), np.asarray(inp['off_b'], np.float32)
    for g in range(G):
        for p_ in range(P):
            wox[:, g * 9 + p_] = ow[g * 18 + p_ * 2 + 0]
            woy[:, g * 9 + p_] = ow[g * 18 + p_ * 2 + 1]
            box[g * 9 + p_, 0] = ob[g * 18 + p_ * 2 + 0]
            boy[g * 9 + p_, 0] = ob[g * 18 + p_ * 2 + 1]
    w['wox'], w['woy'], w['box'], w['boy'] = wox, woy, box, boy
    w['wmk'] = np.ascontiguousarray(inp['mask_w'].T).astype(np.float32)  # [256,36]
    import ml_dtypes as _mld
    for k in ('wox', 'woy', 'wmk'):
        w[k] = w[k].astype(_mld.bfloat16)
    w['bmk'] = inp['mask_b'].reshape(36, 1).astype(np.float32)
    w['wout'] = np.ascontiguousarray(inp['out_w'].T).astype(np.float32)  # [gc,o]
    w['bout'] = inp['out_b'].reshape(C, 1).astype(np.float32)
    Smats, E9, E9T = _host_consts()
    import ml_dtypes
    w['smats'] = np.ascontiguousarray(Smats.reshape(9, 36, 100)).astype(ml_dtypes.bfloat16)
    w['e9'], w['e9t'] = E9.astype(ml_dtypes.bfloat16), E9T
    e8 = np.zeros((8, 8, 128), np.float32)
    for n in range(8):
        e8[n, n, :] = 1.0
    w['e8sel'] = e8.reshape(8, 1024)
    return w


def _build(nc, tc, have_inb):
    import concourse.bass as bass
    import concourse.mybir as mybir
    from concourse.masks import make_identity
    f32 = mybir.dt.float32
    bf16 = mybir.dt.bfloat16
    AF = mybir.ActivationFunctionType
    ALU = mybir.AluOpType

    def dram(name, shape, dt=f32, kind="ExternalInput"):
        return nc.dram_tensor(name, shape, dt, kind=kind).ap()

    x_d = dram("x", [C_IN, S])
    wc_d = dram("wc", [C_IN, C])
    bc_d = dram("bc", [C, 1])
    win_d = dram("win", [C, C], bf16)
    dwd_d = dram("dwd", [128, 9, 2, 128], bf16)
    bdw_d = dram("bdw", [C, 1])
    lng_d = dram("lng", [C, 1])
    lnb_d = dram("lnb", [C, 1])
    wox_d = dram("wox", [C, 36], bf16)
    woy_d = dram("woy", [C, 36], bf16)
    wmk_d = dram("wmk", [C, 36], bf16)
    box_d = dram("box", [36, 1])
    boy_d = dram("boy", [36, 1])
    bmk_d = dram("bmk", [36, 1])
    wout_d = dram("wout", [C, C])
    bout_d = dram("bout", [C, 1])
    S_d = dram("smats", [9, 36, 100], bf16)
    e9_d = dram("e9", [36, 4], bf16)
    e9t_d = dram("e9t", [4, 36])
    e8_d = dram("e8sel", [8, 1024])
    inb_d = dram("inb", [1, C]) if have_inb else None
    out_d = dram("out", [C, S], kind="ExternalOutput")

    def load(pool, dr, shape, dt=f32, tag=None):
        t = pool.tile(shape, dt, tag=tag, name=tag)
        nc.sync.dma_start(out=t, in_=dr)
        return t

    def flat(t):
        return t.rearrange("p a b -> p (a b)")

    NB = 8          # n-blocks of 512
    CH = 4          # FMA oy-chunks
    CHH = H // CH   # 16 rows per chunk

    with tc.tile_pool(name="consts", bufs=1) as consts:
        wc = [load(consts, wc_d[0:128, :], [128, C], tag="wc0"),
              load(consts, wc_d[128:192, :], [64, C], tag="wc1")]
        bc = [load(consts, bc_d[0:128], [128, 1], tag="bc0"),
              load(consts, bc_d[128:256], [128, 1], tag="bc1")]
        win = [load(consts, win_d[0:128, :], [128, C], bf16, tag="win0"),
               load(consts, win_d[128:256, :], [128, C], bf16, tag="win1")]
        dwd = load(consts, dwd_d, [128, 9, 2, 128], bf16, tag="dwd")
        bdw = [load(consts, bdw_d[0:128], [128, 1], tag="bdw0"),
               load(consts, bdw_d[128:256], [128, 1], tag="bdw1")]
        lng = [load(consts, lng_d[0:128], [128, 1], tag="lng0"),
               load(consts, lng_d[128:256], [128, 1], tag="lng1")]
        lnb = [load(consts, lnb_d[0:128], [128, 1], tag="lnb0"),
               load(consts, lnb_d[128:256], [128, 1], tag="lnb1")]
        wox = [load(consts, wox_d[0:128, :], [128, 36], bf16, tag="wox0"),
               load(consts, wox_d[128:256, :], [128, 36], bf16, tag="wox1")]
        woy = [load(consts, woy_d[0:128, :], [128, 36], bf16, tag="woy0"),
               load(consts, woy_d[128:256, :], [128, 36], bf16, tag="woy1")]
        wmk = [load(consts, wmk_d[0:128, :], [128, 36], bf16, tag="wmk0"),
               load(consts, wmk_d[128:256, :], [128, 36], bf16, tag="wmk1")]
        box = load(consts, box_d, [36, 1], tag="box")
        boy = load(consts, boy_d, [36, 1], tag="boy")
        bmk = load(consts, bmk_d, [36, 1], tag="bmk")
        wout = [load(consts, wout_d[0:128, :], [128, C], tag="wout0"),
                load(consts, wout_d[128:256, :], [128, C], tag="wout1")]
        bout = [load(consts, bout_d[0:128], [128, 1], tag="bout0"),
                load(consts, bout_d[128:256], [128, 1], tag="bout1")]
        smt = [load(consts, S_d[i], [36, 100], bf16, tag=f"smt{i}") for i in range(9)]
        e9 = load(consts, e9_d, [36, 4], bf16, tag="e9")
        e9t = load(consts, e9t_d, [4, 36], tag="e9t")
        e8 = load(consts, e8_d, [8, 8, 128], tag="e8")
        ident = consts.tile([128, 128], f32, tag="ident", name="ident")
        make_identity(nc, ident)
        identb = consts.tile([128, 128], bf16, tag="identb", name="identb")
        make_identity(nc, identb)
        ones_k = consts.tile([128, 1], f32, tag="ones_k", name="ones_k")
        nc.vector.memset(ones_k, 1.0)
        eps8 = consts.tile([8, 1], f32, tag="eps8", name="eps8")
        nc.vector.memset(eps8, 1e-5)
        b_p1 = consts.tile([36, 1], f32, tag="b_p1", name="b_p1")
        nc.vector.memset(b_p1, 1.0)
        b_m1 = consts.tile([36, 1], f32, tag="b_m1", name="b_m1")
        nc.vector.memset(b_m1, -1.0)
        if have_inb:
            inb_b = consts.tile([128, C], f32, tag="inb", name="inb")
            nc.sync.dma_start(out=inb_b, in_=bass.AP(tensor=inb_d.tensor, offset=0,
                                                     ap=[[0, 128], [1, C]]))

        with tc.tile_pool(name="pers", bufs=1) as pers:
            # persistent mid-pipeline tensors
            t_ = [pers.tile([128, H, W], f32, tag=f"t{m}", name=f"t{m}") for m in range(2)]
            # val_T2: partition (h, ox), h = oy//32; free (oy%32, c)
            val_T = pers.tile([128, 32, C], bf16, tag="valT", name="valT")
            DWT = [pers.tile([128, H, 25], f32, tag=f"DWT{pr}", name=f"DWT{pr}")
                   for pr in range(2)]
            acc = [pers.tile([128, H, GC], f32, tag=f"acc{pr}", name=f"acc{pr}")
                   for pr in range(2)]

            with tc.tile_pool(name="psF", bufs=4, space="PSUM") as psF:
                with tc.tile_pool(name="M3", bufs=1) as M3:
                    DW = M3.tile([100, S], bf16, tag="DW", name="DW")
                    tbuf = M3.tile([128, 32, 100], f32, tag="tbuf", name="tbuf")

                    with tc.tile_pool(name="M1", bufs=1) as M1:
                        y = [M1.tile([128, H, W], bf16, tag=f"y{m}", name=f"y{m}")
                             for m in range(2)]
                        ypad = [M1.tile([128, 66, 66], bf16, tag=f"yp{m}", name=f"yp{m}")
                                for m in range(2)]

                        # ---- 1x1 conv (x streamed in 512-col slices) ----
                        with tc.tile_pool(name="xsP", bufs=3) as xsP:
                            for n in range(NB):
                                sl = slice(n * 512, (n + 1) * 512)
                                xs0 = load(xsP, x_d[0:128, sl], [128, 512], tag="xs0")
                                xs1 = load(xsP, x_d[128:192, sl], [64, 512], tag="xs1")
                                for mt in range(2):
                                    ps = psF.tile([128, 512], f32, tag="ps", name="ps")
                                    nc.tensor.matmul(ps, wc[0][:, mt * 128:(mt + 1) * 128], xs0, start=True, stop=False)
                                    nc.tensor.matmul(ps, wc[1][:, mt * 128:(mt + 1) * 128], xs1, start=False, stop=True)
                                    nc.scalar.activation(flat(y[mt])[:, sl], ps, AF.Identity, bias=bc[mt])

                        # ---- in_proj -> val_T2 (two oy-halves via psum col halves) ----
                        for oy in range(H):
                            h = oy // 32
                            ps = psF.tile([128, C], f32, tag="ps", name="ps")
                            po = ps[h * 64:(h + 1) * 64, :]
                            nc.tensor.matmul(po, y[0][:, oy, :], win[0], start=True, stop=False)
                            nc.tensor.matmul(po, y[1][:, oy, :], win[1], start=False, stop=True)
                            nc.scalar.activation(val_T[h * 64:(h + 1) * 64, oy % 32, :], po, AF.Identity)
                        if have_inb:
                            bcast = bass.AP(tensor=inb_b.tensor, offset=inb_b.offset,
                                            ap=[inb_b.ap[0], [0, 32], [1, C]])
                            nc.vector.tensor_add(val_T, val_T, bcast)

                        # ---- ypad + depthwise conv -> t ----
                        for mt in range(2):
                            nc.gpsimd.memset(ypad[mt], 0.0)
                            nc.vector.tensor_copy(ypad[mt][:, 1:65, 1:65], y[mt])
                        for mt in range(2):
                            for n in range(NB):
                                ps = psF.tile([128, 8, 64], f32, tag="ps", name="ps")
                                oy0 = n * 8
                                for tap in range(9):
                                    ky, kx = tap // 3, tap % 3
                                    nc.tensor.matmul(ps, dwd[:, tap, mt, :],
                                                     ypad[mt][:, oy0 + ky:oy0 + ky + 8, kx:kx + 64],
                                                     start=(tap == 0), stop=(tap == 8))
                                nc.scalar.activation(t_[mt][:, oy0:oy0 + 8, :], ps, AF.Identity, bias=bdw[mt])

                    # ---- M2: LN stats + normalize + offsets/masks + DW build ----
                    with tc.tile_pool(name="M2", bufs=1) as M2:
                        sA = M2.tile([8, 512], f32, tag="sA", name="sA")   # sum t -> mean -> mv
                        sB = M2.tile([8, 512], f32, tag="sB", name="sB")   # sum t2 -> var -> 1/var
                        sC = M2.tile([8, 512], f32, tag="sC", name="sC")   # mean^2
                        sD = M2.tile([8, 512], f32, tag="sD", name="sD")   # minv
                        with tc.tile_pool(name="sqP", bufs=3) as sqP:
                            for (isq, dst8) in ((0, sA), (1, sB)):
                                for n in range(NB):
                                    sl = slice(n * 512, (n + 1) * 512)
                                    ps = psF.tile([1, 512], f32, tag="ps", name="ps")
                                    if isq:
                                        for mt in range(2):
                                            tq = sqP.tile([128, 512], f32, tag="tq", name="tq")
                                            nc.scalar.activation(tq, flat(t_[mt])[:, sl], AF.Square)
                                            nc.tensor.matmul(ps, ones_k, tq, start=(mt == 0), stop=(mt == 1))
                                    else:
                                        nc.tensor.matmul(ps, ones_k, flat(t_[0])[:, sl], start=True, stop=False)
                                        nc.tensor.matmul(ps, ones_k, flat(t_[1])[:, sl], start=False, stop=True)
                                    stg = sqP.tile([1, 512], f32, tag="stg", name="stg")
                                    nc.vector.tensor_copy(stg, ps)
                                    nc.sync.dma_start(out=dst8[n:n + 1, :], in_=stg)
                        nc.scalar.mul(sA, sA, 1.0 / C)
                        nc.scalar.mul(sB, sB, 1.0 / C)
                        nc.scalar.activation(sC, sA, AF.Square)
                        nc.vector.scalar_tensor_tensor(sB, sC, -1.0, sB, op0=ALU.mult, op1=ALU.add)
                        nc.scalar.activation(sB, sB, AF.Identity, bias=eps8)
                        nc.vector.reciprocal(sB, sB)
                        nc.scalar.activation(sD, sB, AF.Sqrt)
                        nc.vector.tensor_mul(sA, sA, sD)

                        # normalize + gelu -> ta (bf16)
                        ta = [M2.tile([128, H, W], bf16, tag=f"ta{m}", name=f"ta{m}")
                              for m in range(2)]
                        with tc.tile_pool(name="uP", bufs=3) as uP:
                            for n in range(NB):
                                sl = slice(n * 512, (n + 1) * 512)
                                ps1 = psF.tile([128, 512], f32, tag="ps", name="ps")
                                ps2 = psF.tile([128, 512], f32, tag="ps", name="ps")
                                nc.tensor.matmul(ps1, e8[:, n, :], sD, start=True, stop=True)
                                nc.tensor.matmul(ps2, e8[:, n, :], sA, start=True, stop=True)
                                for mt in range(2):
                                    u = uP.tile([128, 512], f32, tag="u", name="u")
                                    nc.vector.tensor_mul(u, flat(t_[mt])[:, sl], ps1)
                                    nc.vector.tensor_sub(u, u, ps2)
                                    nc.scalar.activation(flat(ta[mt])[:, sl], u, AF.Gelu, bias=lnb[mt], scale=lng[mt])

                        # offsets/masks/hats/DW, s-chunked (4 chunks of 1024)
                        SC = 1024
                        for sc in range(4):
                            oxt = M2.tile([36, SC], bf16, tag="oxt", name="oxt")
                            oyt = M2.tile([36, SC], bf16, tag="oyt", name="oyt")
                            ex = M2.tile([36, SC], bf16, tag="ex", name="ex")
                            for nb2 in range(2):
                                n = sc * 2 + nb2
                                sl = slice(n * 512, (n + 1) * 512)
                                cl = slice(nb2 * 512, (nb2 + 1) * 512)
                                for (wgt, bia, dst2, fn) in ((wox, box, oxt, AF.Identity),
                                                             (woy, boy, oyt, AF.Identity),
                                                             (wmk, bmk, ex, AF.Exp)):
                                    ps = psF.tile([36, 512], f32, tag="ps", name="ps")
                                    nc.tensor.matmul(ps, wgt[0], flat(ta[0])[:, sl], start=True, stop=False)
                                    nc.tensor.matmul(ps, wgt[1], flat(ta[1])[:, sl], start=False, stop=True)
                                    nc.scalar.activation(dst2[:, cl], ps, fn, bias=bia)
                            rm = M2.tile([4, SC], f32, tag="rm", name="rm")
                            mask = M2.tile([36, SC], bf16, tag="mask", name="mask")
                            for nb2 in range(2):
                                cl = slice(nb2 * 512, (nb2 + 1) * 512)
                                ps = psF.tile([4, 512], f32, tag="ps", name="ps")
                                nc.tensor.matmul(ps, e9, ex[:, cl], start=True, stop=True)
                                nc.vector.reciprocal(rm[:, cl], ps)
                            for nb2 in range(2):
                                cl = slice(nb2 * 512, (nb2 + 1) * 512)
                                ps = psF.tile([36, 512], f32, tag="ps", name="ps")
                                nc.tensor.matmul(ps, e9t, rm[:, cl], start=True, stop=True)
                                nc.vector.tensor_mul(mask[:, cl], ex[:, cl], ps)

                            def hats(src2, pfx):
                                out3 = []
                                for (kk, off) in (("m", b_p1), ("c", None), ("p", b_m1)):
                                    ab = M2.tile([36, SC], bf16, tag="hab", name="hab")
                                    if off is None:
                                        nc.scalar.activation(ab, src2, AF.Abs)
                                    else:
                                        nc.scalar.activation(ab, src2, AF.Abs, bias=off)
                                    h = M2.tile([36, SC], bf16, tag=f"h{pfx}{kk}", name=f"h{pfx}{kk}")
                                    nc.scalar.activation(h, ab, AF.Relu, bias=b_p1, scale=-1.0)
                                    out3.append(h)
                                return out3
                            hx3 = hats(oxt, "x")
                            hy3 = hats(oyt, "y")
                            for yb in range(3):
                                nc.vector.tensor_mul(hy3[yb], mask, hy3[yb])  # hy -> m1 in place
                            psds = [psF.tile([100, 512], f32, tag=f"dwp{i}",
                                             name=f"dwp{i}", bufs=1) for i in range(2)]
                            for xb in range(3):
                                for yb in range(3):
                                    ki = xb * 3 + yb
                                    txb = M2.tile([36, SC], bf16, tag="txb", name="txb")
                                    nc.vector.tensor_mul(txb, hy3[yb], hx3[xb])
                                    for nb2 in range(2):
                                        cl = slice(nb2 * 512, (nb2 + 1) * 512)
                                        nc.tensor.matmul(psds[nb2], smt[ki], txb[:, cl],
                                                         start=(ki == 0), stop=(ki == 8))
                            for nb2 in range(2):
                                n = sc * 2 + nb2
                                nc.scalar.activation(DW[:, n * 512:(n + 1) * 512], psds[nb2], AF.Identity)

                    # ---- DW_T via PE transposes + remap DMA ----
                    for tch in range(32):
                        ps = psF.tile([128, 100], bf16, tag="ptr", name="ptr", bufs=2)
                        nc.tensor.transpose(ps, DW[:, tch * 128:(tch + 1) * 128], identb[0:100, 0:100])
                        nc.vector.tensor_copy(tbuf[:, tch, :], ps)
                    for pr in range(2):
                        for g2 in range(2):
                            g = pr * 2 + g2
                            for par in range(2):
                                d0 = DWT[pr][g2 * 64:(g2 + 1) * 64, :, :]
                                dst = bass.AP(tensor=d0.tensor, offset=d0.offset + par * 25,
                                              ap=[d0.ap[0], [50, 32], [1, 25]])
                                s0 = tbuf[par * 64:(par + 1) * 64, :, :]
                                src = bass.AP(tensor=s0.tensor, offset=s0.offset + g * 25,
                                              ap=[s0.ap[0], [100, 32], [1, 25]])
                                nc.sync.dma_start(out=dst, in_=src)

                # ---- FMA: 25-bin window sum, chunked over oy ----
                with tc.tile_pool(name="vxP", bufs=2) as vxP:
                    for ci in range(CH):
                        oy0 = ci * CHH
                        vxc = [[vxP.tile([128, CHH + 4, GC], bf16, tag=f"vx{pr}_{dxi}",
                                         name=f"vx{pr}_{dxi}")
                                for dxi in range(5)] for pr in range(2)]
                        for pr in range(2):
                            for dxi in range(5):
                                dx = dxi - 2
                                nc.gpsimd.memset(vxc[pr][dxi], 0.0)
                                # interior: vy = global val row; local iy idx = vy + 2 - oy0
                                vy_lo = max(0, oy0 - 2)
                                vy_hi = min(H, oy0 + CHH + 2)
                                for g2 in range(2):
                                    g = pr * 2 + g2
                                    lo = max(0, -dx)
                                    hi = min(64, 64 - dx)
                                    # split at the h-boundary (val row 32)
                                    for (a, b) in ((vy_lo, min(vy_hi, 32)), (max(vy_lo, 32), vy_hi)):
                                        if a >= b:
                                            continue
                                        h = a // 32
                                        dst = vxc[pr][dxi][g2 * 64 + lo:g2 * 64 + hi,
                                                           a + 2 - oy0:b + 2 - oy0, :]
                                        src = val_T[h * 64 + lo + dx:h * 64 + hi + dx,
                                                    a - h * 32:b - h * 32,
                                                    g * GC:(g + 1) * GC]
                                        nc.sync.dma_start(out=dst, in_=src)
                        for pr in range(2):
                            for oyl in range(CHH):
                                oy = oy0 + oyl
                                eng = nc.vector
                                first = True
                                for dyi in range(5):
                                    for dxi in range(5):
                                        d = dyi * 5 + dxi
                                        sc = DWT[pr][:, oy, d:d + 1]
                                        v = vxc[pr][dxi][:, oyl + dyi, :]
                                        o = acc[pr][:, oy, :]
                                        if first:
                                            eng.tensor_scalar_mul(o, v, sc)
                                            first = False
                                        else:
                                            eng.scalar_tensor_tensor(o, v, sc, o, op0=ALU.mult, op1=ALU.add)

            # ---- transpose acc back + out_proj ----
            with tc.tile_pool(name="psT", bufs=4, space="PSUM") as psT:
                with tc.tile_pool(name="E1", bufs=1) as E1:
                    RO = [E1.tile([128, H, W], f32, tag=f"ro{pr}", name=f"ro{pr}")
                          for pr in range(2)]
                    tb2 = E1.tile([128, 32, 128], f32, tag="tb2", name="tb2")
                    for pr in range(2):
                        for tch in range(32):
                            ps = psT.tile([128, 128], f32, tag="ps", name="ps")
                            nc.tensor.transpose(ps, flat(acc[pr])[:, tch * 128:(tch + 1) * 128], ident)
                            nc.scalar.activation(tb2[:, tch, :], ps, AF.Identity)
                        for g2 in range(2):
                            for par in range(2):
                                d0 = RO[pr][g2 * 64:(g2 + 1) * 64, :, :]
                                dst = bass.AP(tensor=d0.tensor, offset=d0.offset + par * 64,
                                              ap=[d0.ap[0], [128, 32], [1, 64]])
                                s0 = tb2[par * 64:(par + 1) * 64, :, :]
                                src = bass.AP(tensor=s0.tensor, offset=s0.offset + g2 * 64,
                                              ap=[s0.ap[0], [128, 32], [1, 64]])
                                nc.sync.dma_start(out=dst, in_=src)

                    for mt in range(2):
                        for n in range(NB):
                            sl = slice(n * 512, (n + 1) * 512)
                            ps = psT.tile([128, 512], f32, tag="ops", name="ops")
                            nc.tensor.matmul(ps, wout[0][:, mt * 128:(mt + 1) * 128],
                                             flat(RO[0])[:, sl], start=True, stop=False)
                            nc.tensor.matmul(ps, wout[1][:, mt * 128:(mt + 1) * 128],
                                             flat(RO[1])[:, sl], start=False, stop=True)
                            osb = E1.tile([128, 512], f32, tag="osb", name="osb", bufs=3)
                            nc.scalar.activation(osb, ps, AF.Identity, bias=bout[mt])
                            nc.sync.dma_start(out=out_d[mt * 128:(mt + 1) * 128, sl], in_=osb)


def _get_program(have_inb):
    key = ("prog", have_inb)
    if key not in _CACHE:
        import concourse.bacc as bacc
        import concourse.tile as tile
        nc = bacc.Bacc("TRN2", target_bir_lowering=False, debug=False,
                       enable_asserts=False)
        with tile.TileContext(nc) as tc:
            _build(nc, tc, have_inb)
        nc.compile()
        _CACHE[key] = nc
    return _CACHE[key]


def kernel(**inputs):
    inputs = {k: np.asarray(v) for k, v in inputs.items()}
    w = _prep_weights(inputs)
    have_inb = bool(np.any(w['inb']))
    nc = _get_program(have_inb)

    base = {
        'wc': w['wc'], 'bc': w['bc'], 'win': w['win'], 'dwd': w['dwd'],
        'bdw': w['bdw'], 'lng': w['ln_g'], 'lnb': w['ln_b'],
        'wox': w['wox'], 'woy': w['woy'], 'wmk': w['wmk'],
        'box': w['box'], 'boy': w['boy'], 'bmk': w['bmk'],
        'wout': w['wout'], 'bout': w['bout'],
        'smats': w['smats'], 'e9': w['e9'], 'e9t': w['e9t'], 'e8sel': w['e8sel'],
    }
    if have_inb:
        base['inb'] = w['inb'].reshape(1, C)
    x = np.asarray(inputs['x'], np.float32).reshape(N, C_IN, S)
    in_maps = []
    for core in range(NCORES):
        m = dict(base)
        m['x'] = np.ascontiguousarray(x[core])
        in_maps.append(m)

    from concourse import bass_utils
    res = bass_utils.run_bass_kernel_spmd(nc, in_maps, core_ids=list(range(NCORES)),
                                          trace=TRACE)
    global _LAST_EXEC_NS
    _LAST_EXEC_NS = res.exec_time_ns
    if TRACE:
        import sys
        print(f"[kernel] exec_time_ns={res.exec_time_ns} trace={res.instructions_and_trace[1] if res.instructions_and_trace else None}", file=sys.stderr)
    out = np.stack([r['out'].reshape(C, H, W) for r in res.results])
    return out.astype(np.float32)



# revision 30
# speedup vs baseline: 1.5504x; 1.5504x over previous
"""DCNv3_C Trainium2 Bass kernel.

8-core data parallelism over the batch (one image per NeuronCore).
Per core: 1x1 conv -> value proj -> depthwise 3x3 (block-diag matmuls)
-> LN+gelu -> offset/mask proj -> softmax -> 25-bin window weights
(hat products + one-hot matmuls) -> window FMA (DVE/Pool per-row
multiplies + PE identity-matmul PSUM accumulation) -> output proj.

DCNv3 bilinear sampling is rewritten exactly (for |offset|<=1) as a 5x5
locally-connected window:
  acc[s,g,c] = sum_{dy,dx in [-2,2]} DW[s,g,dy,dx] * VP[s+(dy,dx), g, c]
  DW[s,g,dy,dx] = sum_p mask_p * haty_p(dy) * hatx_p(dx)
with the three hat taps per axis decomposed as
  (hat(o+1), hat(o), hat(o-1)) = (relu(-o), 1-relu(o)-relu(-o), relu(o)).

The middle of the network is chunked (8-row blocks for the DW build,
16-row blocks for the window FMA and output tail) so everything
pipelines; DMA instruction count is kept low (packed constants, merged
transfers) because each DMA costs ~625ns on the shared hardware DGE.
"""

import numpy as np

N, C_IN, C, H, W = 8, 192, 256, 64, 64
G, K, PAD = 4, 3, 1
GC = C // G          # 64
P = K * K            # 9
S = H * W            # 4096
NCORES = 8

NSC = 8              # DW-build sub-chunks (8 rows / 512 px)
SR = H // NSC        # 8
SC = SR * W          # 512
FCH = 4              # FMA chunks (16 rows / 1024 px)
FR = H // FCH        # 16
FS = FR * W          # 1024

# FMA multiply row split: rows with (idx % 8) < DVE_SHARE go to DVE,
# rest to gpsimd.
DVE_SHARE = 6

_CACHE = {}
TRACE = False
_LAST_EXEC_NS = None


def _host_consts():
    import ml_dtypes
    bf = ml_dtypes.bfloat16
    # p = a*3+b with grid_x = a-1 (slowest), grid_y = b-1
    gx = np.repeat(np.arange(3) - 1, 3)
    gy = np.tile(np.arange(3) - 1, 3)
    # p-sum selection matrices, one per (xb, yb): [36, 100]
    # row (g, p) -> col g*25 + d, d = (dy+2)*5 + (dx+2)
    Smats = np.zeros((3, 3, 36, 100), np.float32)
    for xb in range(3):
        for yb in range(3):
            for g in range(G):
                for p_ in range(P):
                    dy = gy[p_] + yb - 1
                    dx = gx[p_] + xb - 1
                    d = (dy + 2) * 5 + (dx + 2)
                    Smats[xb, yb, g * 9 + p_, g * 25 + d] = 1.0
    E9 = np.zeros((36, 4), np.float32)     # per-group sums
    E9T = np.zeros((4, 36), np.float32)    # per-group broadcast
    for g in range(G):
        E9[g * 9:(g + 1) * 9, g] = 1.0
        E9T[g, g * 9:(g + 1) * 9] = 1.0
    out = {}
    out["smats"] = np.ascontiguousarray(Smats.reshape(9, 36, 100).transpose(1, 0, 2)).astype(bf)
    out["e9"] = np.ascontiguousarray(E9).astype(bf)
    out["e9t"] = np.ascontiguousarray(E9T).astype(np.float32)
    return out


def _prep_weights(inp):
    import ml_dtypes
    bf = ml_dtypes.bfloat16
    f16 = np.float16
    w = {}
    w['wc'] = np.ascontiguousarray(inp['conv_w'].T).astype(bf)            # [192,256]
    # packed per-partition biases/scales [128, 10]:
    # (bc0,bc1,bdw0,bdw1,lng0,lng1,lnb0,lnb1,bout0,bout1)
    bp = np.zeros((128, 10), np.float32)
    for i, vec in enumerate((inp['conv_b'], inp['dw_b'], inp['ln_g'],
                             inp['ln_b'], inp['out_b'])):
        v = np.asarray(vec, np.float32)
        bp[:, 2 * i] = v[0:128]
        bp[:, 2 * i + 1] = v[128:256]
    w['biaspack'] = bp
    w['inb'] = np.asarray(inp['in_b'], np.float32)
    # win/wout packed [128, 2, 256] (half-major lhsT)
    win = np.ascontiguousarray(inp['in_w'].T).astype(np.float32)
    winp = np.zeros((128, 2, 256), np.float32)
    winp[:, 0, :] = win[0:128]
    winp[:, 1, :] = win[128:256]
    w['winp'] = winp.astype(bf)
    wout = np.ascontiguousarray(inp['out_w'].T).astype(np.float32)
    woutp = np.zeros((128, 2, 256), np.float32)
    woutp[:, 0, :] = wout[0:128]
    woutp[:, 1, :] = wout[128:256]
    w['woutp'] = woutp.astype(f16)
    # depthwise diag weights, partition-major: [128, 9, 2, 128]
    dwd = np.zeros((128, 9, 2, 128), np.float32)
    dw = inp['dw_w'].reshape(C, 9)
    for tap in range(9):
        for mt in range(2):
            for i in range(128):
                dwd[i, tap, mt, i] = dw[mt * 128 + i, tap]
    w['dwd'] = dwd.astype(bf)
    # offset/mask projections packed [128, 2, 3, 36], kind = (ox, oy, mk)
    wox = np.zeros((C, 36), np.float32)
    woy = np.zeros((C, 36), np.float32)
    b36 = np.zeros((36, 4), np.float32)
    ow, ob = np.asarray(inp['off_w'], np.float32), np.asarray(inp['off_b'], np.float32)
    for g in range(G):
        for p_ in range(P):
            wox[:, g * 9 + p_] = ow[g * 18 + p_ * 2 + 0]
            woy[:, g * 9 + p_] = ow[g * 18 + p_ * 2 + 1]
            b36[g * 9 + p_, 0] = ob[g * 18 + p_ * 2 + 0]
            b36[g * 9 + p_, 1] = ob[g * 18 + p_ * 2 + 1]
    b36[:, 2] = np.asarray(inp['mask_b'], np.float32)
    b36[:, 3] = 1.0
    wmk = np.ascontiguousarray(inp['mask_w'].T).astype(np.float32)
    wproj = np.zeros((128, 2, 3, 36), np.float32)
    for h in range(2):
        sl = slice(h * 128, (h + 1) * 128)
        wproj[:, h, 0, :] = wox[sl]
        wproj[:, h, 1, :] = woy[sl]
        wproj[:, h, 2, :] = wmk[sl]
    w['wproj'] = wproj.astype(bf)
    w['b36'] = b36
    w.update(_host_consts())
    return w


def _build(nc, tc, have_inb):
    import concourse.bass as bass
    import concourse.mybir as mybir
    from concourse.masks import make_identity
    f32 = mybir.dt.float32
    bf16 = mybir.dt.bfloat16
    fp16 = mybir.dt.float16
    AF = mybir.ActivationFunctionType
    ALU = mybir.AluOpType

    def dram(name, shape, dt=f32, kind="ExternalInput"):
        return nc.dram_tensor(name, shape, dt, kind=kind).ap()

    x_d = dram("x", [C_IN, S], bf16)
    wc_d = dram("wc", [C_IN, C], bf16)
    bp_d = dram("biaspack", [128, 10])
    winp_d = dram("winp", [128, 2, C], bf16)
    woutp_d = dram("woutp", [128, 2, C], fp16)
    dwd_d = dram("dwd", [128, 9, 2, 128], bf16)
    wproj_d = dram("wproj", [128, 2, 3, 36], bf16)
    b36_d = dram("b36", [36, 4])
    S_d = dram("smats", [36, 9, 100], bf16)
    e9_d = dram("e9", [36, 4], bf16)
    e9t_d = dram("e9t", [4, 36])
    inb_d = dram("inb", [1, C]) if have_inb else None
    smv_d = dram("smv_scratch", [2, 8, 512], bf16, kind="Internal")
    out_d = dram("out", [C, S], kind="ExternalOutput")

    def load(pool, dr, shape, dt=f32, tag=None):
        t = pool.tile(shape, dt, tag=tag, name=tag)
        nc.sync.dma_start(out=t, in_=dr)
        return t

    def flat(t):
        return t.rearrange("p a b -> p (a b)")

    with tc.tile_pool(name="consts", bufs=1) as consts:
        wc = [load(consts, wc_d[0:128, :], [128, C], bf16, tag="wc0"),
              load(consts, wc_d[128:192, :], [64, C], bf16, tag="wc1")]
        bp = load(consts, bp_d, [128, 10], tag="bp")
        bc = [bp[:, 0:1], bp[:, 1:2]]
        bdw = [bp[:, 2:3], bp[:, 3:4]]
        lng = [bp[:, 4:5], bp[:, 5:6]]
        lnb = [bp[:, 6:7], bp[:, 7:8]]
        bout = [bp[:, 8:9], bp[:, 9:10]]
        winp = load(consts, winp_d, [128, 2, C], bf16, tag="winp")
        win = [winp[:, 0, :], winp[:, 1, :]]
        woutp = load(consts, woutp_d, [128, 2, C], fp16, tag="woutp")
        wout = [woutp[:, 0, :], woutp[:, 1, :]]
        dwd = load(consts, dwd_d, [128, 9, 2, 128], bf16, tag="dwd")
        wproj = load(consts, wproj_d, [128, 2, 3, 36], bf16, tag="wproj")
        wox = [wproj[:, 0, 0, :], wproj[:, 1, 0, :]]
        woy = [wproj[:, 0, 1, :], wproj[:, 1, 1, :]]
        wmk = [wproj[:, 0, 2, :], wproj[:, 1, 2, :]]
        b36 = load(consts, b36_d, [36, 4], tag="b36")
        box, boy, bmk, one36 = b36[:, 0:1], b36[:, 1:2], b36[:, 2:3], b36[:, 3:4]
        smt_t = load(consts, S_d, [36, 9, 100], bf16, tag="smt")
        smt = [smt_t[:, i, :] for i in range(9)]
        e9 = load(consts, e9_d, [36, 4], bf16, tag="e9")
        e9t = load(consts, e9t_d, [4, 36], tag="e9t")
        identb = consts.tile([128, 128], bf16, tag="identb", name="identb")
        make_identity(nc, identb)
        identh = consts.tile([128, 128], fp16, tag="identh", name="identh")
        make_identity(nc, identh)
        ones_k = consts.tile([128, 1], bf16, tag="ones_k", name="ones_k")
        nc.vector.memset(ones_k, 1.0 / C)
        if have_inb:
            inb_b = consts.tile([128, C], f32, tag="inb", name="inb")
            nc.sync.dma_start(out=inb_b, in_=bass.AP(tensor=inb_d.tensor, offset=0,
                                                     ap=[[0, 128], [1, C]]))

        with tc.tile_pool(name="pers", bufs=1) as pers:
            t_ = [pers.tile([128, H, W], bf16, tag=f"t{m}", name=f"t{m}") for m in range(2)]
            # val_T: partition (h, ox), h = oy//32; free (oy%32, c)
            val_T = pers.tile([128, 32, C], bf16, tag="valT", name="valT")
            sA = pers.tile([8, 512], f32, tag="sA", name="sA")   # mean
            sB = pers.tile([8, 512], f32, tag="sB", name="sB")   # E[t^2] -> var
            sDb = pers.tile([8, 512], bf16, tag="sDb", name="sDb")  # 1/sigma
            mvb = pers.tile([8, 512], bf16, tag="mvb", name="mvb")  # mean/sigma

            with tc.tile_pool(name="psA", bufs=2, space="PSUM") as psA, \
                 tc.tile_pool(name="psAcc", bufs=4, space="PSUM") as psAcc, \
                 tc.tile_pool(name="psTr", bufs=2, space="PSUM") as psTr, \
                 tc.tile_pool(name="work", bufs=2) as wk, \
                 tc.tile_pool(name="xsP", bufs=2) as xsP, \
                 tc.tile_pool(name="tmpP", bufs=6) as tmpP, \
                 tc.tile_pool(name="vxP", bufs=2) as vxP, \
                 tc.tile_pool(name="osbP", bufs=2) as osbP:

                with tc.tile_pool(name="ypadP", bufs=1) as ypadP:
                    ypad = [ypadP.tile([128, 66, 66], bf16, tag=f"yp{m}", name=f"yp{m}")
                            for m in range(2)]
                    for mt in range(2):
                        nc.gpsimd.memset(ypad[mt][:, 0, :], 0.0)
                        nc.gpsimd.memset(ypad[mt][:, 65, :], 0.0)
                        nc.gpsimd.memset(ypad[mt][:, 1:65, 0:1], 0.0)
                        nc.gpsimd.memset(ypad[mt][:, 1:65, 65:66], 0.0)

                    # ---- 1x1 conv writes ypad interior directly ----
                    for nn in range(4):
                        fsl = slice(nn * FS, (nn + 1) * FS)
                        xs0 = load(xsP, x_d[0:128, fsl], [128, FS], bf16, tag="xs0")
                        xs1 = load(xsP, x_d[128:192, fsl], [64, FS], bf16, tag="xs1")
                        for j in range(2):
                            n = nn * 2 + j
                            jl = slice(j * SC, (j + 1) * SC)
                            for mt in range(2):
                                psf = psA.tile([128, SC], f32, tag="ps", name="ps")
                                nc.tensor.matmul(psf, wc[0][:, mt * 128:(mt + 1) * 128],
                                                 xs0[:, jl], start=True, stop=False)
                                nc.tensor.matmul(psf, wc[1][:, mt * 128:(mt + 1) * 128],
                                                 xs1[:, jl], start=False, stop=True)
                                ps = psf.rearrange("p (a b) -> p a b", a=SR)
                                nc.scalar.activation(ypad[mt][:, 1 + n * SR:1 + (n + 1) * SR, 1:65],
                                                     ps, AF.Identity, bias=bc[mt])

                    # ---- in_proj -> val_T (two oy-halves via psum partition halves) ----
                    for oy in range(H):
                        h = oy // 32
                        ps = psA.tile([128, SC], f32, tag="ps", name="ps")
                        po = ps[h * 64:(h + 1) * 64, 0:C]
                        nc.tensor.matmul(po, ypad[0][:, 1 + oy, 1:65], win[0], start=True, stop=False)
                        nc.tensor.matmul(po, ypad[1][:, 1 + oy, 1:65], win[1], start=False, stop=True)
                        nc.scalar.activation(val_T[h * 64:(h + 1) * 64, oy % 32, :], po, AF.Identity)
                    if have_inb:
                        bcast = bass.AP(tensor=inb_b.tensor, offset=inb_b.offset,
                                        ap=[inb_b.ap[0], [0, 32], [1, C]])
                        nc.vector.tensor_add(val_T, val_T, bcast)

                    # ---- depthwise conv -> t ----
                    for mt in range(2):
                        for n in range(NSC):
                            psf = psA.tile([128, SC], f32, tag="ps", name="ps")
                            ps = psf.rearrange("p (a b) -> p a b", a=SR)
                            oy0 = n * SR
                            for tap in range(9):
                                ky, kx = tap // 3, tap % 3
                                nc.tensor.matmul(ps, dwd[:, tap, mt, :],
                                                 ypad[mt][:, oy0 + ky:oy0 + ky + SR, kx:kx + 64],
                                                 start=(tap == 0), stop=(tap == 8))
                            nc.scalar.activation(t_[mt][:, oy0:oy0 + SR, :], ps, AF.Identity,
                                                 bias=bdw[mt])

                # ---- LN stats (batched over all sub-chunks) ----
                for n in range(NSC):
                    sl = slice(n * SC, (n + 1) * SC)
                    psf = psA.tile([128, SC], f32, tag="ps", name="ps")
                    ps = psf[0:1, :]
                    nc.tensor.matmul(ps, ones_k, flat(t_[0])[:, sl], start=True, stop=False)
                    nc.tensor.matmul(ps, ones_k, flat(t_[1])[:, sl], start=False, stop=True)
                    stg = wk.tile([1, SC], f32, tag="stg", name="stg", bufs=1)
                    nc.scalar.activation(stg, ps, AF.Identity)
                    nc.sync.dma_start(out=sA[n:n + 1, :], in_=stg)
                for n in range(NSC):
                    sl = slice(n * SC, (n + 1) * SC)
                    psf = psA.tile([128, SC], f32, tag="ps", name="ps")
                    ps = psf[0:1, :]
                    for mt in range(2):
                        tq = wk.tile([128, SC], bf16, tag="tq", name="tq")
                        nc.scalar.activation(tq, flat(t_[mt])[:, sl], AF.Square)
                        nc.tensor.matmul(ps, ones_k, tq, start=(mt == 0), stop=(mt == 1))
                    stg = wk.tile([1, SC], f32, tag="stg", name="stg", bufs=1)
                    nc.scalar.activation(stg, ps, AF.Identity)
                    nc.sync.dma_start(out=sB[n:n + 1, :], in_=stg)
                sC_ = pers.tile([8, 512], f32, tag="sC", name="sC")
                nc.vector.tensor_mul(sC_, sA, sA)
                nc.vector.tensor_sub(sB, sB, sC_)
                nc.vector.tensor_scalar_add(sB, sB, 1e-5)
                nc.vector.reciprocal_approx_fast(out=sC_, in_=sB)
                nc.scalar.activation(sDb, sC_, AF.Sqrt)
                nc.vector.tensor_mul(mvb, sA, sDb)
                nc.sync.dma_start(out=smv_d[0], in_=sDb)
                nc.sync.dma_start(out=smv_d[1], in_=mvb)

                # ---- main chunk loop (FMA chunks of 16 rows) ----
                for ci in range(FCH):
                    oy0 = ci * FR

                    # vxc loads (shifted/padded value windows for the FMA)
                    vxc = [[vxP.tile([128, FR + 4, GC], bf16, tag=f"vx{pr}_{dxi}",
                                     name=f"vx{pr}_{dxi}")
                            for dxi in range(5)] for pr in range(2)]
                    vy_lo = max(0, oy0 - 2)
                    vy_hi = min(H, oy0 + FR + 2)
                    for pr in range(2):
                        for dxi in range(5):
                            dx = dxi - 2
                            if ci < 2:
                                nc.gpsimd.memset(vxc[pr][dxi], 0.0)
                            elif ci == FCH - 1:
                                nc.gpsimd.memset(vxc[pr][dxi][:, FR + 2:, :], 0.0)
                            for g2 in range(2):
                                g = pr * 2 + g2
                                lo = max(0, -dx)
                                hi = min(64, 64 - dx)
                                for (a, b) in ((vy_lo, min(vy_hi, 32)), (max(vy_lo, 32), vy_hi)):
                                    if a >= b:
                                        continue
                                    h = a // 32
                                    dst = vxc[pr][dxi][g2 * 64 + lo:g2 * 64 + hi,
                                                       a + 2 - oy0:b + 2 - oy0, :]
                                    src = val_T[h * 64 + lo + dx:h * 64 + hi + dx,
                                                a - h * 32:b - h * 32,
                                                g * GC:(g + 1) * GC]
                                    nc.sync.dma_start(out=dst, in_=src)

                    # DW build over 2 sub-chunks of 8 rows
                    tb = wk.tile([128, 8, 100], f32, tag="tb", name="tb", bufs=1)
                    for sc2 in range(2):
                        scg = ci * 2 + sc2
                        cl = slice(scg * SC, (scg + 1) * SC)

                        # normalize + gelu (stats broadcast via stride-0 DRAM DMA)
                        sdmv = wk.tile([128, 2, SC], bf16, tag="sdmv", name="sdmv")
                        nc.sync.dma_start(out=sdmv, in_=bass.AP(tensor=smv_d.tensor,
                                                                offset=scg * SC,
                                                                ap=[[0, 128], [4096, 2], [1, SC]]))
                        ta = []
                        for mt in range(2):
                            u = wk.tile([128, SC], bf16, tag="u", name="u")
                            nc.vector.tensor_mul(u, flat(t_[mt])[:, cl], sdmv[:, 0, :])
                            nc.vector.tensor_sub(u, u, sdmv[:, 1, :])
                            tam = wk.tile([128, SC], bf16, tag=f"ta{mt}", name=f"ta{mt}")
                            nc.scalar.activation(tam, u, AF.Gelu, bias=lnb[mt], scale=lng[mt])
                            ta.append(tam)

                        # offset/mask projections
                        oxt = wk.tile([36, SC], bf16, tag="oxt", name="oxt", bufs=1)
                        oyt = wk.tile([36, SC], bf16, tag="oyt", name="oyt", bufs=1)
                        ex = wk.tile([36, SC], bf16, tag="ex", name="ex", bufs=1)
                        for (wgt, bia, dst2, fn) in ((wox, box, oxt, AF.Identity),
                                                     (woy, boy, oyt, AF.Identity),
                                                     (wmk, bmk, ex, AF.Exp)):
                            psf = psA.tile([128, SC], f32, tag="ps", name="ps")
                            ps = psf[0:36, :]
                            nc.tensor.matmul(ps, wgt[0], ta[0], start=True, stop=False)
                            nc.tensor.matmul(ps, wgt[1], ta[1], start=False, stop=True)
                            nc.scalar.activation(dst2, ps, fn, bias=bia)

                        # softmax: mask = ex / (per-group sum of ex)
                        psf4 = psA.tile([128, SC], f32, tag="ps", name="ps")
                        ps4 = psf4[0:4, :]
                        nc.tensor.matmul(ps4, e9, ex, start=True, stop=True)
                        rmf = wk.tile([4, SC], f32, tag="rmf", name="rmf", bufs=1)
                        nc.vector.reciprocal_approx_fast(out=rmf, in_=ps4)
                        psmf = psA.tile([128, SC], f32, tag="ps", name="ps")
                        psm = psmf[0:36, :]
                        nc.tensor.matmul(psm, e9t, rmf, start=True, stop=True)
                        mask = wk.tile([36, SC], bf16, tag="mask", name="mask", bufs=1)
                        nc.vector.tensor_mul(mask, ex, psm)

                        # hats via relu (|o|<=1): hp = relu(o), hm = hp - o,
                        # hc = 1 - hp - hm
                        hpx = wk.tile([36, SC], bf16, tag="hpx", name="hpx", bufs=1)
                        hmx = wk.tile([36, SC], bf16, tag="hmx", name="hmx", bufs=1)
                        hcx = wk.tile([36, SC], bf16, tag="hcx", name="hcx", bufs=1)
                        nc.vector.tensor_scalar_max(hpx, oxt, 0.0)
                        nc.vector.tensor_sub(hmx, hpx, oxt)
                        nc.vector.tensor_add(hcx, hpx, hmx)
                        nc.scalar.activation(hcx, hcx, AF.Identity, bias=one36, scale=-1.0)
                        hpy = wk.tile([36, SC], bf16, tag="hpy", name="hpy", bufs=1)
                        hmy = wk.tile([36, SC], bf16, tag="hmy", name="hmy", bufs=1)
                        nc.vector.tensor_scalar_max(hpy, oyt, 0.0)
                        nc.vector.tensor_sub(hmy, hpy, oyt)
                        # mask-folded y-hats: m1[yb] = mask * hy[yb]
                        m1p = wk.tile([36, SC], bf16, tag="m1p", name="m1p", bufs=1)
                        m1m = wk.tile([36, SC], bf16, tag="m1m", name="m1m", bufs=1)
                        m1c = wk.tile([36, SC], bf16, tag="m1c", name="m1c", bufs=1)
                        nc.vector.tensor_mul(m1p, mask, hpy)
                        nc.vector.tensor_mul(m1m, mask, hmy)
                        nc.vector.tensor_sub(m1c, mask, m1m)
                        nc.vector.tensor_sub(m1c, m1c, m1p)
                        hx3 = [hmx, hcx, hpx]
                        my3 = [m1m, m1c, m1p]

                        # DW = sum_(xb,yb) smats[xb*3+yb]^T (my3[yb] * hx3[xb])
                        psDWf = psA.tile([128, SC], f32, tag="ps", name="ps")
                        psDW = psDWf[0:100, :]
                        for xb in range(3):
                            for yb in range(3):
                                ki = xb * 3 + yb
                                txb = wk.tile([36, SC], bf16, tag="txb", name="txb", bufs=3)
                                nc.vector.tensor_mul(txb, my3[yb], hx3[xb])
                                nc.tensor.matmul(psDW, smt[ki], txb,
                                                 start=(ki == 0), stop=(ki == 8))
                        DW = wk.tile([100, SC], bf16, tag="DW", name="DW")
                        nc.scalar.activation(DW, psDW, AF.Identity)

                        # transpose DW -> tb [128=(s%128), 8, 100]
                        for tch in range(4):
                            psf = psTr.tile([128, 128], bf16, tag="ptr", name="ptr")
                            ps = psf[:, 0:100]
                            nc.tensor.transpose(ps, DW[:, tch * 128:(tch + 1) * 128],
                                                identb[0:100, 0:100])
                            nc.scalar.activation(tb[:, sc2 * 4 + tch, :], ps, AF.Identity)

                    # remap to DWT[pr]: partition (g2, ox), free (oyl, d)
                    DWT = [wk.tile([128, FR, 25], f32, tag=f"DWT{pr}", name=f"DWT{pr}")
                           for pr in range(2)]
                    for pr in range(2):
                        for g2 in range(2):
                            g = pr * 2 + g2
                            for par in range(2):
                                d0 = DWT[pr][g2 * 64:(g2 + 1) * 64, :, :]
                                dst = bass.AP(tensor=d0.tensor, offset=d0.offset + par * 25,
                                              ap=[d0.ap[0], [50, 8], [1, 25]])
                                s0 = tb[par * 64:(par + 1) * 64, :, :]
                                src = bass.AP(tensor=s0.tensor, offset=s0.offset + g * 25,
                                              ap=[s0.ap[0], [100, 8], [1, 25]])
                                nc.sync.dma_start(out=dst, in_=src)

                    # ---- FMA: DVE/Pool multiplies + PE identity accumulate ----
                    acc = []
                    for pr in range(2):
                        acc_ps = [psAcc.tile([128, SR, GC], f32, tag="accps", name="accps")
                                  for _ in range(2)]
                        for d in range(25):
                            dyi, dxi = d // 5, d % 5
                            tmp = tmpP.tile([128, FR, GC], bf16, tag="tmp", name="tmp")
                            for oyl in range(FR):
                                eng = nc.vector if ((oyl + pr * 4) % 8) < DVE_SHARE else nc.gpsimd
                                eng.tensor_scalar_mul(tmp[:, oyl, :],
                                                      vxc[pr][dxi][:, oyl + dyi, :],
                                                      DWT[pr][:, oyl, d:d + 1])
                            for hh in range(2):
                                nc.tensor.matmul(flat(acc_ps[hh]), identb,
                                                 flat(tmp)[:, hh * SC:(hh + 1) * SC],
                                                 start=(d == 0), stop=(d == 24))
                        am = wk.tile([128, FR, GC], fp16, tag=f"acc{pr}", name=f"acc{pr}", bufs=1)
                        for hh in range(2):
                            nc.scalar.activation(am[:, hh * SR:(hh + 1) * SR, :],
                                                 acc_ps[hh], AF.Identity)
                        acc.append(am)

                    # ---- transpose acc -> RO (c on partitions) ----
                    RO = []
                    for pr in range(2):
                        tb2 = wk.tile([128, 8, 128], fp16, tag=f"tb2_{pr}", name=f"tb2_{pr}", bufs=1)
                        for tch in range(8):
                            psf = psTr.tile([128, 128], fp16, tag="ptr", name="ptr")
                            nc.tensor.transpose(psf, flat(acc[pr])[:, tch * 128:(tch + 1) * 128],
                                                identh)
                            nc.scalar.activation(tb2[:, tch, :], psf, AF.Identity)
                        rom = wk.tile([128, FR, W], fp16, tag=f"ro{pr}", name=f"ro{pr}")
                        for g2 in range(2):
                            for par in range(2):
                                d0 = rom[g2 * 64:(g2 + 1) * 64, :, :]
                                dst = bass.AP(tensor=d0.tensor, offset=d0.offset + par * 64,
                                              ap=[d0.ap[0], [128, 8], [1, 64]])
                                s0 = tb2[par * 64:(par + 1) * 64, :, :]
                                src = bass.AP(tensor=s0.tensor, offset=s0.offset + g2 * 64,
                                              ap=[s0.ap[0], [128, 8], [1, 64]])
                                nc.sync.dma_start(out=dst, in_=src)
                        RO.append(rom)

                    # ---- output projection ----
                    for mt in range(2):
                        osb = osbP.tile([128, FS], f32, tag="osb", name="osb")
                        for j in range(2):
                            jl = slice(j * SC, (j + 1) * SC)
                            ps = psA.tile([128, SC], f32, tag="ps", name="ps")
                            nc.tensor.matmul(ps, wout[0][:, mt * 128:(mt + 1) * 128],
                                             flat(RO[0])[:, jl], start=True, stop=False)
                            nc.tensor.matmul(ps, wout[1][:, mt * 128:(mt + 1) * 128],
                                             flat(RO[1])[:, jl], start=False, stop=True)
                            nc.scalar.activation(osb[:, jl], ps, AF.Identity, bias=bout[mt])
                        nc.sync.dma_start(out=out_d[mt * 128:(mt + 1) * 128,
                                                    ci * FS:(ci + 1) * FS], in_=osb)


def _get_program(have_inb):
    key = ("prog", have_inb)
    if key not in _CACHE:
        import concourse.bacc as bacc
        import concourse.tile as tile
        nc = bacc.Bacc("TRN2", target_bir_lowering=False, debug=False,
                       enable_asserts=False)
        with tile.TileContext(nc) as tc:
            _build(nc, tc, have_inb)
        nc.compile()
        _CACHE[key] = nc
    return _CACHE[key]


def kernel(**inputs):
    import ml_dtypes
    inputs = {k: np.asarray(v) for k, v in inputs.items()}
    w = _prep_weights(inputs)
    have_inb = bool(np.any(w['inb']))
    nc = _get_program(have_inb)

    base = {
        'wc': w['wc'], 'biaspack': w['biaspack'], 'winp': w['winp'],
        'woutp': w['woutp'], 'dwd': w['dwd'], 'wproj': w['wproj'],
        'b36': w['b36'], 'smats': w['smats'], 'e9': w['e9'], 'e9t': w['e9t'],
    }
    if have_inb:
        base['inb'] = w['inb'].reshape(1, C)
    x = np.asarray(inputs['x'], np.float32).reshape(N, C_IN, S).astype(ml_dtypes.bfloat16)
    in_maps = []
    for core in range(NCORES):
        m = dict(base)
        m['x'] = np.ascontiguousarray(x[core])
        in_maps.append(m)

    from concourse import bass_utils
    res = bass_utils.run_bass_kernel_spmd(nc, in_maps, core_ids=list(range(NCORES)),
                                          trace=TRACE)
    global _LAST_EXEC_NS
    _LAST_EXEC_NS = res.exec_time_ns
    if TRACE:
        import sys
        print(f"[kernel] exec_time_ns={res.exec_time_ns} trace={res.instructions_and_trace[1] if res.instructions_and_trace else None}", file=sys.stderr)
    out = np.stack([r['out'].reshape(C, H, W) for r in res.results])
    return out.astype(np.float32)


# revision 31
# speedup vs baseline: 1.5963x; 1.0296x over previous
"""DCNv3_C Trainium2 Bass kernel.

8-core data parallelism over the batch (one image per NeuronCore).
Per core: 1x1 conv -> value proj -> depthwise 3x3 (block-diag matmuls)
-> LN+gelu -> offset/mask proj -> softmax -> 25-bin window weights
(hat products + one-hot matmuls) -> window FMA (DVE/Pool per-row
multiplies + PE identity-matmul PSUM accumulation) -> output proj.

DCNv3 bilinear sampling is rewritten exactly (for |offset|<=1) as a 5x5
locally-connected window:
  acc[s,g,c] = sum_{dy,dx in [-2,2]} DW[s,g,dy,dx] * VP[s+(dy,dx), g, c]
  DW[s,g,dy,dx] = sum_p mask_p * haty_p(dy) * hatx_p(dx)
with the three hat taps per axis decomposed as
  (hat(o+1), hat(o), hat(o-1)) = (relu(-o), 1-relu(o)-relu(-o), relu(o)).

The middle of the network is chunked (8-row blocks for the DW build,
16-row blocks for the window FMA and output tail) so everything
pipelines; DMA instruction count is kept low (packed constants, merged
transfers) because each DMA costs ~625ns on the shared hardware DGE.
"""

import numpy as np

N, C_IN, C, H, W = 8, 192, 256, 64, 64
G, K, PAD = 4, 3, 1
GC = C // G          # 64
P = K * K            # 9
S = H * W            # 4096
NCORES = 8

NSC = 8              # DW-build sub-chunks (8 rows / 512 px)
SR = H // NSC        # 8
SC = SR * W          # 512
FCH = 4              # FMA chunks (16 rows / 1024 px)
FR = H // FCH        # 16
FS = FR * W          # 1024

# FMA multiply row split: rows with (idx % 8) < DVE_SHARE go to DVE,
# rest to gpsimd.
DVE_SHARE = 6

_CACHE = {}
TRACE = False
_LAST_EXEC_NS = None


def _host_consts():
    import ml_dtypes
    bf = ml_dtypes.bfloat16
    # p = a*3+b with grid_x = a-1 (slowest), grid_y = b-1
    gx = np.repeat(np.arange(3) - 1, 3)
    gy = np.tile(np.arange(3) - 1, 3)
    # p-sum selection matrices, one per (xb, yb): [36, 100]
    # row (g, p) -> col g*25 + d, d = (dy+2)*5 + (dx+2)
    Smats = np.zeros((3, 3, 36, 100), np.float32)
    for xb in range(3):
        for yb in range(3):
            for g in range(G):
                for p_ in range(P):
                    dy = gy[p_] + yb - 1
                    dx = gx[p_] + xb - 1
                    d = (dy + 2) * 5 + (dx + 2)
                    Smats[xb, yb, g * 9 + p_, g * 25 + d] = 1.0
    E9 = np.zeros((36, 4), np.float32)     # per-group sums
    E9T = np.zeros((4, 36), np.float32)    # per-group broadcast
    for g in range(G):
        E9[g * 9:(g + 1) * 9, g] = 1.0
        E9T[g, g * 9:(g + 1) * 9] = 1.0
    out = {}
    out["smats"] = np.ascontiguousarray(Smats.reshape(9, 36, 100).transpose(1, 0, 2)).astype(bf)
    out["e9"] = np.ascontiguousarray(E9).astype(bf)
    out["e9t"] = np.ascontiguousarray(E9T).astype(np.float32)
    return out


def _prep_weights(inp):
    import ml_dtypes
    bf = ml_dtypes.bfloat16
    f16 = np.float16
    w = {}
    w['wc'] = np.ascontiguousarray(inp['conv_w'].T).astype(bf)            # [192,256]
    # packed per-partition biases/scales [128, 10]:
    # (bc0,bc1,bdw0,bdw1,lng0,lng1,lnb0,lnb1,bout0,bout1)
    bp = np.zeros((128, 10), np.float32)
    for i, vec in enumerate((inp['conv_b'], inp['dw_b'], inp['ln_g'],
                             inp['ln_b'], inp['out_b'])):
        v = np.asarray(vec, np.float32)
        bp[:, 2 * i] = v[0:128]
        bp[:, 2 * i + 1] = v[128:256]
    w['biaspack'] = bp
    w['inb'] = np.asarray(inp['in_b'], np.float32)
    # win/wout packed [128, 2, 256] (half-major lhsT)
    win = np.ascontiguousarray(inp['in_w'].T).astype(np.float32)
    winp = np.zeros((128, 2, 256), np.float32)
    winp[:, 0, :] = win[0:128]
    winp[:, 1, :] = win[128:256]
    w['winp'] = winp.astype(bf)
    wout = np.ascontiguousarray(inp['out_w'].T).astype(np.float32)
    woutp = np.zeros((128, 2, 256), np.float32)
    woutp[:, 0, :] = wout[0:128]
    woutp[:, 1, :] = wout[128:256]
    w['woutp'] = woutp.astype(f16)
    # depthwise diag weights, partition-major: [128, 9, 2, 128]
    dwd = np.zeros((128, 9, 2, 128), np.float32)
    dw = inp['dw_w'].reshape(C, 9)
    for tap in range(9):
        for mt in range(2):
            for i in range(128):
                dwd[i, tap, mt, i] = dw[mt * 128 + i, tap]
    w['dwd'] = dwd.astype(bf)
    # offset/mask projections packed [128, 2, 3, 36], kind = (ox, oy, mk)
    wox = np.zeros((C, 36), np.float32)
    woy = np.zeros((C, 36), np.float32)
    b36 = np.zeros((36, 4), np.float32)
    ow, ob = np.asarray(inp['off_w'], np.float32), np.asarray(inp['off_b'], np.float32)
    for g in range(G):
        for p_ in range(P):
            wox[:, g * 9 + p_] = ow[g * 18 + p_ * 2 + 0]
            woy[:, g * 9 + p_] = ow[g * 18 + p_ * 2 + 1]
            b36[g * 9 + p_, 0] = ob[g * 18 + p_ * 2 + 0]
            b36[g * 9 + p_, 1] = ob[g * 18 + p_ * 2 + 1]
    b36[:, 2] = np.asarray(inp['mask_b'], np.float32)
    b36[:, 3] = 1.0
    wmk = np.ascontiguousarray(inp['mask_w'].T).astype(np.float32)
    wproj = np.zeros((128, 2, 3, 36), np.float32)
    for h in range(2):
        sl = slice(h * 128, (h + 1) * 128)
        wproj[:, h, 0, :] = wox[sl]
        wproj[:, h, 1, :] = woy[sl]
        wproj[:, h, 2, :] = wmk[sl]
    w['wproj'] = wproj.astype(bf)
    w['b36'] = b36
    w.update(_host_consts())
    return w


def _build(nc, tc, have_inb):
    import concourse.bass as bass
    import concourse.mybir as mybir
    from concourse.masks import make_identity
    f32 = mybir.dt.float32
    bf16 = mybir.dt.bfloat16
    fp16 = mybir.dt.float16
    AF = mybir.ActivationFunctionType
    ALU = mybir.AluOpType

    def dram(name, shape, dt=f32, kind="ExternalInput"):
        return nc.dram_tensor(name, shape, dt, kind=kind).ap()

    x_d = dram("x", [C_IN, S], bf16)
    wc_d = dram("wc", [C_IN, C], bf16)
    bp_d = dram("biaspack", [128, 10])
    winp_d = dram("winp", [128, 2, C], bf16)
    woutp_d = dram("woutp", [128, 2, C], fp16)
    dwd_d = dram("dwd", [128, 9, 2, 128], bf16)
    wproj_d = dram("wproj", [128, 2, 3, 36], bf16)
    b36_d = dram("b36", [36, 4])
    S_d = dram("smats", [36, 9, 100], bf16)
    e9_d = dram("e9", [36, 4], bf16)
    e9t_d = dram("e9t", [4, 36])
    inb_d = dram("inb", [1, C]) if have_inb else None
    smv_d = dram("smv_scratch", [2, 8, 512], bf16, kind="Internal")
    out_d = dram("out", [C, S], kind="ExternalOutput")

    def load(pool, dr, shape, dt=f32, tag=None):
        t = pool.tile(shape, dt, tag=tag, name=tag)
        nc.sync.dma_start(out=t, in_=dr)
        return t

    def flat(t):
        return t.rearrange("p a b -> p (a b)")

    with tc.tile_pool(name="consts", bufs=1) as consts:
        wc = [load(consts, wc_d[0:128, :], [128, C], bf16, tag="wc0"),
              load(consts, wc_d[128:192, :], [64, C], bf16, tag="wc1")]
        bp = load(consts, bp_d, [128, 10], tag="bp")
        bc = [bp[:, 0:1], bp[:, 1:2]]
        bdw = [bp[:, 2:3], bp[:, 3:4]]
        lng = [bp[:, 4:5], bp[:, 5:6]]
        lnb = [bp[:, 6:7], bp[:, 7:8]]
        bout = [bp[:, 8:9], bp[:, 9:10]]
        winp = load(consts, winp_d, [128, 2, C], bf16, tag="winp")
        win = [winp[:, 0, :], winp[:, 1, :]]
        woutp = load(consts, woutp_d, [128, 2, C], fp16, tag="woutp")
        wout = [woutp[:, 0, :], woutp[:, 1, :]]
        dwd = load(consts, dwd_d, [128, 9, 2, 128], bf16, tag="dwd")
        wproj = load(consts, wproj_d, [128, 2, 3, 36], bf16, tag="wproj")
        wox = [wproj[:, 0, 0, :], wproj[:, 1, 0, :]]
        woy = [wproj[:, 0, 1, :], wproj[:, 1, 1, :]]
        wmk = [wproj[:, 0, 2, :], wproj[:, 1, 2, :]]
        b36 = load(consts, b36_d, [36, 4], tag="b36")
        box, boy, bmk, one36 = b36[:, 0:1], b36[:, 1:2], b36[:, 2:3], b36[:, 3:4]
        smt_t = load(consts, S_d, [36, 9, 100], bf16, tag="smt")
        smt = [smt_t[:, i, :] for i in range(9)]
        e9 = load(consts, e9_d, [36, 4], bf16, tag="e9")
        e9t = load(consts, e9t_d, [4, 36], tag="e9t")
        identb = consts.tile([128, 128], bf16, tag="identb", name="identb")
        make_identity(nc, identb)
        identh = consts.tile([128, 128], fp16, tag="identh", name="identh")
        make_identity(nc, identh)
        ones_k = consts.tile([128, 1], bf16, tag="ones_k", name="ones_k")
        nc.vector.memset(ones_k, 1.0 / C)
        if have_inb:
            inb_b = consts.tile([128, C], f32, tag="inb", name="inb")
            nc.sync.dma_start(out=inb_b, in_=bass.AP(tensor=inb_d.tensor, offset=0,
                                                     ap=[[0, 128], [1, C]]))

        with tc.tile_pool(name="pers", bufs=1) as pers:
            t_ = [pers.tile([128, H, W], bf16, tag=f"t{m}", name=f"t{m}") for m in range(2)]
            # val_T: partition (h, ox), h = oy//32; free (oy%32, c)
            val_T = pers.tile([128, 32, C], bf16, tag="valT", name="valT")
            sA = pers.tile([8, 512], f32, tag="sA", name="sA")   # mean
            sB = pers.tile([8, 512], f32, tag="sB", name="sB")   # E[t^2] -> var
            sDb = pers.tile([8, 512], bf16, tag="sDb", name="sDb")  # 1/sigma
            mvb = pers.tile([8, 512], bf16, tag="mvb", name="mvb")  # mean/sigma

            with tc.tile_pool(name="psA", bufs=4, space="PSUM") as psA, \
                 tc.tile_pool(name="psAcc", bufs=2, space="PSUM") as psAcc, \
                 tc.tile_pool(name="psTr", bufs=2, space="PSUM") as psTr, \
                 tc.tile_pool(name="work", bufs=2) as wk, \
                 tc.tile_pool(name="xsP", bufs=2) as xsP, \
                 tc.tile_pool(name="tmpP", bufs=6) as tmpP, \
                 tc.tile_pool(name="vxP", bufs=2) as vxP, \
                 tc.tile_pool(name="osbP", bufs=2) as osbP:

                with tc.tile_pool(name="ypadP", bufs=1) as ypadP:
                    ypad = [ypadP.tile([128, 66, 66], bf16, tag=f"yp{m}", name=f"yp{m}")
                            for m in range(2)]
                    for mt in range(2):
                        nc.gpsimd.memset(ypad[mt][:, 0, :], 0.0)
                        nc.gpsimd.memset(ypad[mt][:, 65, :], 0.0)
                        nc.gpsimd.memset(ypad[mt][:, 1:65, 0:1], 0.0)
                        nc.gpsimd.memset(ypad[mt][:, 1:65, 65:66], 0.0)

                    # ---- 1x1 conv writes ypad interior directly ----
                    for nn in range(4):
                        fsl = slice(nn * FS, (nn + 1) * FS)
                        xs0 = load(xsP, x_d[0:128, fsl], [128, FS], bf16, tag="xs0")
                        xs1 = load(xsP, x_d[128:192, fsl], [64, FS], bf16, tag="xs1")
                        for j in range(2):
                            n = nn * 2 + j
                            jl = slice(j * SC, (j + 1) * SC)
                            for mt in range(2):
                                psf = psA.tile([128, SC], f32, tag="ps", name="ps")
                                nc.tensor.matmul(psf, wc[0][:, mt * 128:(mt + 1) * 128],
                                                 xs0[:, jl], start=True, stop=False)
                                nc.tensor.matmul(psf, wc[1][:, mt * 128:(mt + 1) * 128],
                                                 xs1[:, jl], start=False, stop=True)
                                ps = psf.rearrange("p (a b) -> p a b", a=SR)
                                nc.scalar.activation(ypad[mt][:, 1 + n * SR:1 + (n + 1) * SR, 1:65],
                                                     ps, AF.Identity, bias=bc[mt])

                    # ---- in_proj -> val_T (two oy-halves via psum partition halves) ----
                    for oy in range(H):
                        h = oy // 32
                        ps = psA.tile([128, SC], f32, tag="ps", name="ps")
                        po = ps[h * 64:(h + 1) * 64, 0:C]
                        nc.tensor.matmul(po, ypad[0][:, 1 + oy, 1:65], win[0], start=True, stop=False)
                        nc.tensor.matmul(po, ypad[1][:, 1 + oy, 1:65], win[1], start=False, stop=True)
                        nc.scalar.activation(val_T[h * 64:(h + 1) * 64, oy % 32, :], po, AF.Identity)
                    if have_inb:
                        bcast = bass.AP(tensor=inb_b.tensor, offset=inb_b.offset,
                                        ap=[inb_b.ap[0], [0, 32], [1, C]])
                        nc.vector.tensor_add(val_T, val_T, bcast)

                    # ---- depthwise conv -> t ----
                    for mt in range(2):
                        for n in range(NSC):
                            psf = psA.tile([128, SC], f32, tag="ps", name="ps")
                            ps = psf.rearrange("p (a b) -> p a b", a=SR)
                            oy0 = n * SR
                            for tap in range(9):
                                ky, kx = tap // 3, tap % 3
                                nc.tensor.matmul(ps, dwd[:, tap, mt, :],
                                                 ypad[mt][:, oy0 + ky:oy0 + ky + SR, kx:kx + 64],
                                                 start=(tap == 0), stop=(tap == 8))
                            nc.scalar.activation(t_[mt][:, oy0:oy0 + SR, :], ps, AF.Identity,
                                                 bias=bdw[mt])

                # ---- LN stats (batched over all sub-chunks) ----
                for n in range(NSC):
                    sl = slice(n * SC, (n + 1) * SC)
                    psf = psA.tile([128, SC], f32, tag="ps", name="ps")
                    ps = psf[0:1, :]
                    nc.tensor.matmul(ps, ones_k, flat(t_[0])[:, sl], start=True, stop=False)
                    nc.tensor.matmul(ps, ones_k, flat(t_[1])[:, sl], start=False, stop=True)
                    stg = wk.tile([1, SC], f32, tag="stg", name="stg", bufs=1)
                    nc.scalar.activation(stg, ps, AF.Identity)
                    nc.sync.dma_start(out=sA[n:n + 1, :], in_=stg)
                for n in range(NSC):
                    sl = slice(n * SC, (n + 1) * SC)
                    psf = psA.tile([128, SC], f32, tag="ps", name="ps")
                    ps = psf[0:1, :]
                    for mt in range(2):
                        tq = wk.tile([128, SC], bf16, tag="tq", name="tq")
                        nc.scalar.activation(tq, flat(t_[mt])[:, sl], AF.Square)
                        nc.tensor.matmul(ps, ones_k, tq, start=(mt == 0), stop=(mt == 1))
                    stg = wk.tile([1, SC], f32, tag="stg", name="stg", bufs=1)
                    nc.scalar.activation(stg, ps, AF.Identity)
                    nc.sync.dma_start(out=sB[n:n + 1, :], in_=stg)
                sC_ = pers.tile([8, 512], f32, tag="sC", name="sC")
                nc.vector.tensor_mul(sC_, sA, sA)
                nc.vector.tensor_sub(sB, sB, sC_)
                nc.vector.tensor_scalar_add(sB, sB, 1e-5)
                nc.vector.reciprocal_approx_fast(out=sC_, in_=sB)
                nc.scalar.activation(sDb, sC_, AF.Sqrt)
                nc.vector.tensor_mul(mvb, sA, sDb)
                nc.sync.dma_start(out=smv_d[0], in_=sDb)
                nc.sync.dma_start(out=smv_d[1], in_=mvb)

                # ---- main chunk loop (FMA chunks of 16 rows) ----
                for ci in range(FCH):
                    oy0 = ci * FR

                    # vxc loads (shifted/padded value windows for the FMA)
                    vxc = [[vxP.tile([128, FR + 4, GC], bf16, tag=f"vx{pr}_{dxi}",
                                     name=f"vx{pr}_{dxi}")
                            for dxi in range(5)] for pr in range(2)]
                    vy_lo = max(0, oy0 - 2)
                    vy_hi = min(H, oy0 + FR + 2)
                    for pr in range(2):
                        for dxi in range(5):
                            dx = dxi - 2
                            if ci < 2:
                                nc.gpsimd.memset(vxc[pr][dxi], 0.0)
                            elif ci == FCH - 1:
                                nc.gpsimd.memset(vxc[pr][dxi][:, FR + 2:, :], 0.0)
                            for g2 in range(2):
                                g = pr * 2 + g2
                                lo = max(0, -dx)
                                hi = min(64, 64 - dx)
                                for (a, b) in ((vy_lo, min(vy_hi, 32)), (max(vy_lo, 32), vy_hi)):
                                    if a >= b:
                                        continue
                                    h = a // 32
                                    dst = vxc[pr][dxi][g2 * 64 + lo:g2 * 64 + hi,
                                                       a + 2 - oy0:b + 2 - oy0, :]
                                    src = val_T[h * 64 + lo + dx:h * 64 + hi + dx,
                                                a - h * 32:b - h * 32,
                                                g * GC:(g + 1) * GC]
                                    nc.sync.dma_start(out=dst, in_=src)

                    # DW build over 2 sub-chunks of 8 rows
                    tb = wk.tile([128, 8, 100], f32, tag="tb", name="tb", bufs=1)
                    for sc2 in range(2):
                        scg = ci * 2 + sc2
                        cl = slice(scg * SC, (scg + 1) * SC)

                        # normalize + gelu (stats broadcast via stride-0 DRAM DMA)
                        sdmv = wk.tile([128, 2, SC], bf16, tag="sdmv", name="sdmv")
                        nc.sync.dma_start(out=sdmv, in_=bass.AP(tensor=smv_d.tensor,
                                                                offset=scg * SC,
                                                                ap=[[0, 128], [4096, 2], [1, SC]]))
                        ta = []
                        for mt in range(2):
                            u = wk.tile([128, SC], bf16, tag="u", name="u")
                            nc.vector.tensor_mul(u, flat(t_[mt])[:, cl], sdmv[:, 0, :])
                            nc.vector.tensor_sub(u, u, sdmv[:, 1, :])
                            tam = wk.tile([128, SC], bf16, tag=f"ta{mt}", name=f"ta{mt}")
                            nc.scalar.activation(tam, u, AF.Gelu, bias=lnb[mt], scale=lng[mt])
                            ta.append(tam)

                        # offset/mask projections
                        oxt = wk.tile([36, SC], bf16, tag="oxt", name="oxt", bufs=1)
                        oyt = wk.tile([36, SC], bf16, tag="oyt", name="oyt", bufs=1)
                        ex = wk.tile([36, SC], bf16, tag="ex", name="ex", bufs=1)
                        for (wgt, bia, dst2, fn) in ((wox, box, oxt, AF.Identity),
                                                     (woy, boy, oyt, AF.Identity),
                                                     (wmk, bmk, ex, AF.Exp)):
                            psf = psA.tile([128, SC], f32, tag="ps", name="ps")
                            ps = psf[0:36, :]
                            nc.tensor.matmul(ps, wgt[0], ta[0], start=True, stop=False)
                            nc.tensor.matmul(ps, wgt[1], ta[1], start=False, stop=True)
                            nc.scalar.activation(dst2, ps, fn, bias=bia)

                        # softmax: mask = ex / (per-group sum of ex)
                        psf4 = psA.tile([128, SC], f32, tag="ps", name="ps")
                        ps4 = psf4[0:4, :]
                        nc.tensor.matmul(ps4, e9, ex, start=True, stop=True)
                        rmf = wk.tile([4, SC], f32, tag="rmf", name="rmf", bufs=1)
                        nc.vector.reciprocal_approx_fast(out=rmf, in_=ps4)
                        psmf = psA.tile([128, SC], f32, tag="ps", name="ps")
                        psm = psmf[0:36, :]
                        nc.tensor.matmul(psm, e9t, rmf, start=True, stop=True)
                        mask = wk.tile([36, SC], bf16, tag="mask", name="mask", bufs=1)
                        nc.vector.tensor_mul(mask, ex, psm)

                        # hats via relu (|o|<=1): hp = relu(o), hm = hp - o,
                        # hc = 1 - hp - hm
                        hpx = wk.tile([36, SC], bf16, tag="hpx", name="hpx", bufs=1)
                        hmx = wk.tile([36, SC], bf16, tag="hmx", name="hmx", bufs=1)
                        hcx = wk.tile([36, SC], bf16, tag="hcx", name="hcx", bufs=1)
                        nc.vector.tensor_scalar_max(hpx, oxt, 0.0)
                        nc.vector.tensor_sub(hmx, hpx, oxt)
                        nc.vector.tensor_add(hcx, hpx, hmx)
                        nc.scalar.activation(hcx, hcx, AF.Identity, bias=one36, scale=-1.0)
                        hpy = wk.tile([36, SC], bf16, tag="hpy", name="hpy", bufs=1)
                        hmy = wk.tile([36, SC], bf16, tag="hmy", name="hmy", bufs=1)
                        nc.vector.tensor_scalar_max(hpy, oyt, 0.0)
                        nc.vector.tensor_sub(hmy, hpy, oyt)
                        # mask-folded y-hats: m1[yb] = mask * hy[yb]
                        m1p = wk.tile([36, SC], bf16, tag="m1p", name="m1p", bufs=1)
                        m1m = wk.tile([36, SC], bf16, tag="m1m", name="m1m", bufs=1)
                        m1c = wk.tile([36, SC], bf16, tag="m1c", name="m1c", bufs=1)
                        nc.vector.tensor_mul(m1p, mask, hpy)
                        nc.vector.tensor_mul(m1m, mask, hmy)
                        nc.vector.tensor_sub(m1c, mask, m1m)
                        nc.vector.tensor_sub(m1c, m1c, m1p)
                        hx3 = [hmx, hcx, hpx]
                        my3 = [m1m, m1c, m1p]

                        # DW = sum_(xb,yb) smats[xb*3+yb]^T (my3[yb] * hx3[xb])
                        psDWf = psA.tile([128, SC], f32, tag="ps", name="ps")
                        psDW = psDWf[0:100, :]
                        for xb in range(3):
                            for yb in range(3):
                                ki = xb * 3 + yb
                                txb = wk.tile([36, SC], bf16, tag="txb", name="txb", bufs=3)
                                nc.vector.tensor_mul(txb, my3[yb], hx3[xb])
                                nc.tensor.matmul(psDW, smt[ki], txb,
                                                 start=(ki == 0), stop=(ki == 8))
                        DW = wk.tile([100, SC], bf16, tag="DW", name="DW")
                        nc.scalar.activation(DW, psDW, AF.Identity)

                        # transpose DW -> tb [128=(s%128), 8, 100]
                        for tch in range(4):
                            psf = psTr.tile([128, 128], bf16, tag="ptr", name="ptr")
                            ps = psf[:, 0:100]
                            nc.tensor.transpose(ps, DW[:, tch * 128:(tch + 1) * 128],
                                                identb[0:100, 0:100])
                            nc.scalar.activation(tb[:, sc2 * 4 + tch, :], ps, AF.Identity)

                    # remap to DWT[pr]: partition (g2, ox), free (oyl, d)
                    DWT = [wk.tile([128, FR, 25], f32, tag=f"DWT{pr}", name=f"DWT{pr}")
                           for pr in range(2)]
                    for pr in range(2):
                        for g2 in range(2):
                            g = pr * 2 + g2
                            for par in range(2):
                                d0 = DWT[pr][g2 * 64:(g2 + 1) * 64, :, :]
                                dst = bass.AP(tensor=d0.tensor, offset=d0.offset + par * 25,
                                              ap=[d0.ap[0], [50, 8], [1, 25]])
                                s0 = tb[par * 64:(par + 1) * 64, :, :]
                                src = bass.AP(tensor=s0.tensor, offset=s0.offset + g * 25,
                                              ap=[s0.ap[0], [100, 8], [1, 25]])
                                nc.sync.dma_start(out=dst, in_=src)

                    # ---- FMA: DVE/Pool multiplies + PE identity accumulate ----
                    acc = []
                    for pr in range(2):
                        acc_ps = [psAcc.tile([128, SR, GC], f32, tag="accps", name="accps")
                                  for _ in range(2)]
                        for d in range(25):
                            dyi, dxi = d // 5, d % 5
                            tmp = tmpP.tile([128, FR, GC], bf16, tag="tmp", name="tmp")
                            for oyl in range(FR):
                                eng = nc.vector if ((oyl + pr * 4) % 8) < DVE_SHARE else nc.gpsimd
                                eng.tensor_scalar_mul(tmp[:, oyl, :],
                                                      vxc[pr][dxi][:, oyl + dyi, :],
                                                      DWT[pr][:, oyl, d:d + 1])
                            for hh in range(2):
                                nc.tensor.matmul(flat(acc_ps[hh]), identb,
                                                 flat(tmp)[:, hh * SC:(hh + 1) * SC],
                                                 start=(d == 0), stop=(d == 24))
                        am = wk.tile([128, FR, GC], fp16, tag=f"acc{pr}", name=f"acc{pr}", bufs=1)
                        for hh in range(2):
                            nc.scalar.activation(am[:, hh * SR:(hh + 1) * SR, :],
                                                 acc_ps[hh], AF.Identity)
                        acc.append(am)

                    # ---- transpose acc -> RO (c on partitions) ----
                    RO = []
                    for pr in range(2):
                        tb2 = wk.tile([128, 8, 128], fp16, tag=f"tb2_{pr}", name=f"tb2_{pr}", bufs=1)
                        for tch in range(8):
                            psf = psTr.tile([128, 128], fp16, tag="ptr", name="ptr")
                            nc.tensor.transpose(psf, flat(acc[pr])[:, tch * 128:(tch + 1) * 128],
                                                identh)
                            nc.scalar.activation(tb2[:, tch, :], psf, AF.Identity)
                        rom = wk.tile([128, FR, W], fp16, tag=f"ro{pr}", name=f"ro{pr}")
                        for g2 in range(2):
                            for par in range(2):
                                d0 = rom[g2 * 64:(g2 + 1) * 64, :, :]
                                dst = bass.AP(tensor=d0.tensor, offset=d0.offset + par * 64,
                                              ap=[d0.ap[0], [128, 8], [1, 64]])
                                s0 = tb2[par * 64:(par + 1) * 64, :, :]
                                src = bass.AP(tensor=s0.tensor, offset=s0.offset + g2 * 64,
                                              ap=[s0.ap[0], [128, 8], [1, 64]])
                                nc.sync.dma_start(out=dst, in_=src)
                        RO.append(rom)

                    # ---- output projection ----
                    for mt in range(2):
                        osb = osbP.tile([128, FS], f32, tag="osb", name="osb")
                        for j in range(2):
                            jl = slice(j * SC, (j + 1) * SC)
                            ps = psA.tile([128, SC], f32, tag="ps", name="ps")
                            nc.tensor.matmul(ps, wout[0][:, mt * 128:(mt + 1) * 128],
                                             flat(RO[0])[:, jl], start=True, stop=False)
                            nc.tensor.matmul(ps, wout[1][:, mt * 128:(mt + 1) * 128],
                                             flat(RO[1])[:, jl], start=False, stop=True)
                            nc.scalar.activation(osb[:, jl], ps, AF.Identity, bias=bout[mt])
                        nc.sync.dma_start(out=out_d[mt * 128:(mt + 1) * 128,
                                                    ci * FS:(ci + 1) * FS], in_=osb)


def _get_program(have_inb):
    key = ("prog", have_inb)
    if key not in _CACHE:
        import concourse.bacc as bacc
        import concourse.tile as tile
        nc = bacc.Bacc("TRN2", target_bir_lowering=False, debug=False,
                       enable_asserts=False)
        with tile.TileContext(nc) as tc:
            _build(nc, tc, have_inb)
        nc.compile()
        _CACHE[key] = nc
    return _CACHE[key]


def kernel(**inputs):
    import ml_dtypes
    inputs = {k: np.asarray(v) for k, v in inputs.items()}
    w = _prep_weights(inputs)
    have_inb = bool(np.any(w['inb']))
    nc = _get_program(have_inb)

    base = {
        'wc': w['wc'], 'biaspack': w['biaspack'], 'winp': w['winp'],
        'woutp': w['woutp'], 'dwd': w['dwd'], 'wproj': w['wproj'],
        'b36': w['b36'], 'smats': w['smats'], 'e9': w['e9'], 'e9t': w['e9t'],
    }
    if have_inb:
        base['inb'] = w['inb'].reshape(1, C)
    x = np.asarray(inputs['x'], np.float32).reshape(N, C_IN, S).astype(ml_dtypes.bfloat16)
    in_maps = []
    for core in range(NCORES):
        m = dict(base)
        m['x'] = np.ascontiguousarray(x[core])
        in_maps.append(m)

    from concourse import bass_utils
    res = bass_utils.run_bass_kernel_spmd(nc, in_maps, core_ids=list(range(NCORES)),
                                          trace=TRACE)
    global _LAST_EXEC_NS
    _LAST_EXEC_NS = res.exec_time_ns
    if TRACE:
        import sys
        print(f"[kernel] exec_time_ns={res.exec_time_ns} trace={res.instructions_and_trace[1] if res.instructions_and_trace else None}", file=sys.stderr)
    out = np.stack([r['out'].reshape(C, H, W) for r in res.results])
    return out.astype(np.float32)
